# revision 37
# baseline (speedup 1.0000x reference)
"""LoRA Linear (y = x @ W^T + bias + x @ (B@A)^T) on 8 Trainium2 NeuronCores.

Strategy (column-parallel, out_features sharded 8 ways):
  - Each core owns a 512-wide slice of out_features.
  - Split-K mixed precision: the first K8T=16 k-tiles (2048 of 4096
    in_features) run as fp8 e4m3 DoubleRow matmuls (2 k-tiles per
    instruction at 1 cycle/row = 2x bf16 throughput); the remaining 16
    k-tiles run in bf16. Per 128-token chain: 8 DR + 16 bf16 matmuls
    = 24 x 213ns instead of 32 for pure bf16 (~25% less PE time).
    Measured rel err ~1.7e-2 (fp8 quantization noise over half of K),
    within the 2e-2 gate.
  - The LoRA delta is folded into the weight on device before
    quantization: t = W^T + A^T @ B^T (f32); fp8 part: w8 = fp8(t);
    bf16 part: wb = bf16(t), folded in place. The host ships the
    fp8-part weights as bf16 values pre-snapped to the e4m3 grid, so
    the device-side fp8 cast is exact whenever the LoRA delta
    underflows the grid step.
  - W arrives in 4KB-per-partition chunked DMAs (per-k-tile 1KB DMAs
    were packet-overhead-bound and serialized the prologue for ~25us;
    chunking also lets the fold start before the whole 4MB lands).
  - psum layout is [128 tokens, 512 out]; bias is added during PSUM
    eviction; output rows land directly in [tokens, out_shard] layout so
    the host-side gather is a plain concatenate.

Host-side work is layout + dtype encoding only: pack x as [p, T, a, t]
(fp8 plane for the fp8 k-range, bf16 plane for the rest), pre-transpose
W/B slices, broadcast bias; then concatenate the 8 output shards.
"""

import numpy as np
import ml_dtypes

B_DIM, S_DIM = 4, 2048
IN_F = 4096
OUT_F = 4096
RANK = 16
N_CORES = 8
O_SHARD = OUT_F // N_CORES          # 512
TOK = B_DIM * S_DIM                 # 8192
T_TILES = TOK // 128                # 64
K_TILES = IN_F // 128               # 32
K8T = 16                            # k-tiles computed in fp8 (DoubleRow)
KBT = K_TILES - K8T                 # k-tiles computed in bf16
PAIRS = K8T // 2                    # DoubleRow instructions per chain
N_XBUF = 6                          # x-tile pool bufs
W_CHUNK = 4                         # k-tiles per W dma (fold starts early)

_CACHE = {}
LAST_RESULTS = None  # test harness introspection


def _build_nc():
    import concourse.mybir as mybir
    import concourse.tile as tile
    from concourse import bacc

    nc = bacc.Bacc("TRN2", target_bir_lowering=False)
    f32 = mybir.dt.float32
    bf16 = mybir.dt.bfloat16
    f8 = mybir.dt.float8e4
    DR = mybir.MatmulPerfMode.DoubleRow

    x8_d = nc.dram_tensor("x8", (128, T_TILES, K8T, 128), f8,
                          kind="ExternalInput")
    xb_d = nc.dram_tensor("xb", (128, T_TILES, KBT, 128), bf16,
                          kind="ExternalInput")
    w8in_d = nc.dram_tensor("w8in", (128, K8T, O_SHARD), bf16,
                            kind="ExternalInput")
    wb_d = nc.dram_tensor("wb", (128, KBT, O_SHARD), bf16,
                          kind="ExternalInput")
    a_d = nc.dram_tensor("a_t", (RANK, IN_F), bf16, kind="ExternalInput")
    bt_d = nc.dram_tensor("b_t", (RANK, O_SHARD), bf16, kind="ExternalInput")
    bias_d = nc.dram_tensor("bias_b", (128, O_SHARD), f32,
                            kind="ExternalInput")
    y_d = nc.dram_tensor("y", (TOK, O_SHARD), f32, kind="ExternalOutput")

    with tile.TileContext(nc) as tc:
        with (
            tc.tile_pool(name="wpool", bufs=1) as wpool,
            tc.tile_pool(name="wstage", bufs=4) as wstage,
            tc.tile_pool(name="const", bufs=1) as const,
            tc.tile_pool(name="xpool", bufs=N_XBUF) as xpool,
            tc.tile_pool(name="xbpool", bufs=N_XBUF) as xbpool,
            tc.tile_pool(name="opool", bufs=4) as opool,
            tc.tile_pool(name="psum", bufs=4, space="PSUM") as psum_pool,
        ):
            a_sb = const.tile([RANK, IN_F], bf16)
            nc.sync.dma_start(a_sb[:], a_d[:])
            b_sb = const.tile([RANK, O_SHARD], bf16)
            nc.sync.dma_start(b_sb[:], bt_d[:])
            bias_sb = const.tile([128, O_SHARD], f32)
            nc.sync.dma_start(bias_sb[:], bias_d[:])

            # W lands in chunked DMAs (4KB/partition contiguous lines —
            # big enough to avoid packet overhead, small enough that the
            # fold can start before the whole 4MB arrives).
            w8st = wpool.tile([128, K8T, O_SHARD], bf16)
            for c in range(0, K8T, W_CHUNK):
                nc.sync.dma_start(w8st[:, c:c + W_CHUNK, :],
                                  w8in_d[:, c:c + W_CHUNK, :])
            wb_sb = wpool.tile([128, KBT, O_SHARD], bf16)
            for c in range(0, KBT, W_CHUNK):
                nc.sync.dma_start(wb_sb[:, c:c + W_CHUNK, :],
                                  wb_d[:, c:c + W_CHUNK, :])

            # LoRA fold, fp8 k-range (pairs of k-tiles for DoubleRow):
            #   t1 = W^T[k-tile a] + A[:, a*128:(a+1)*128]^T @ B^T   (f32)
            #   w8[pair, slot] = fp8(t1)   (cast on the scalar engine)
            w8_sb = [wpool.tile([128, 2, O_SHARD], f8, tag=f"w8_{p}",
                                name=f"w8_{p}")
                     for p in range(PAIRS)]
            for a in range(K8T):
                p, s = divmod(a, 2)
                pd = psum_pool.tile([128, O_SHARD], f32)
                nc.tensor.matmul(
                    pd[:],
                    a_sb[:, a * 128:(a + 1) * 128],
                    b_sb[:],
                    start=True, stop=True,
                )
                t1 = wstage.tile([128, O_SHARD], f32)
                nc.vector.tensor_add(t1[:], w8st[:, a, :], pd[:])
                nc.scalar.copy(w8_sb[p][:, s, :], t1[:])

            # LoRA fold, bf16 k-range, in place: wb += delta
            for j in range(KBT):
                a = K8T + j
                pd = psum_pool.tile([128, O_SHARD], f32)
                nc.tensor.matmul(
                    pd[:],
                    a_sb[:, a * 128:(a + 1) * 128],
                    b_sb[:],
                    start=True, stop=True,
                )
                nc.vector.tensor_add(wb_sb[:, j, :], wb_sb[:, j, :], pd[:])

            # Main GEMM: psum[128t, 512o] accumulates 8 fp8 DoubleRow
            # matmuls (k-tiles 0..15, two per instruction) + 16 bf16
            # matmuls (k-tiles 16..31) per token tile.
            for t in range(T_TILES):
                x8_sb = xpool.tile([128, K8T, 128], f8)
                nc.sync.dma_start(x8_sb[:], x8_d[:, t, :, :])
                xb_sb = xbpool.tile([128, KBT, 128], bf16)
                nc.sync.dma_start(xb_sb[:], xb_d[:, t, :, :])
                pt = psum_pool.tile([128, O_SHARD], f32)
                for p in range(PAIRS):
                    nc.tensor.matmul(
                        pt[:], x8_sb[:, 2 * p:2 * p + 2, :], w8_sb[p][:],
                        start=(p == 0), stop=False, perf_mode=DR,
                    )
                for j in range(KBT):
                    nc.tensor.matmul(
                        pt[:], xb_sb[:, j, :], wb_sb[:, j, :],
                        start=False, stop=(j == KBT - 1),
                    )
                o_sb = opool.tile([128, O_SHARD], f32)
                nc.vector.tensor_add(o_sb[:], pt[:], bias_sb[:])
                nc.sync.dma_start(y_d[t * 128:(t + 1) * 128, :], o_sb[:])

    nc.compile()
    return nc


def _pack_x(x):
    x2 = np.asarray(x, dtype=np.float32).reshape(TOK, IN_F)
    # x_re[p, T, a, t] = x2[T*128 + t, a*128 + p]
    xr = np.ascontiguousarray(
        x2.reshape(T_TILES, 128, K_TILES, 128).transpose(3, 0, 2, 1))
    x8 = np.ascontiguousarray(xr[:, :, :K8T]).astype(ml_dtypes.float8_e4m3fn)
    xb = np.ascontiguousarray(xr[:, :, K8T:]).astype(ml_dtypes.bfloat16)
    return x8, xb


def kernel(x, weight, A, B, bias):
    global LAST_RESULTS
    from concourse.bass_utils import run_bass_kernel_spmd

    if "nc" not in _CACHE:
        _CACHE["nc"] = _build_nc()
    nc = _CACHE["nc"]

    weight = np.asarray(weight, dtype=np.float32)
    A = np.asarray(A, dtype=np.float32)
    B = np.asarray(B, dtype=np.float32)
    bias = np.asarray(bias, dtype=np.float32)

    x8, xb = _pack_x(x)
    a_t = np.ascontiguousarray(A.astype(ml_dtypes.bfloat16))

    in_maps = []
    for c in range(N_CORES):
        sl = slice(c * O_SHARD, (c + 1) * O_SHARD)
        w_s = weight[sl]                              # (512, 4096)
        # w_re[p, a, o] = w_s[o, a*128 + p]
        w_re = np.ascontiguousarray(
            w_s.T.reshape(K_TILES, 128, O_SHARD).transpose(1, 0, 2))
        # fp8 k-range ships as bf16 snapped to the e4m3 grid (exact in
        # bf16; DVE reads of 8-bit operands measured slower, so the fold
        # consumes bf16 and the scalar engine does the fp8 cast)
        w8in = np.ascontiguousarray(
            w_re[:, :K8T].astype(ml_dtypes.float8_e4m3fn)
            .astype(np.float32).astype(ml_dtypes.bfloat16))
        wb = np.ascontiguousarray(
            w_re[:, K8T:].astype(ml_dtypes.bfloat16))
        b_t = np.ascontiguousarray(B[sl].T.astype(ml_dtypes.bfloat16))
        bias_b = np.ascontiguousarray(
            np.broadcast_to(bias[sl], (128, O_SHARD)))
        in_maps.append({
            "x8": x8,
            "xb": xb,
            "w8in": w8in,
            "wb": wb,
            "a_t": a_t,
            "b_t": b_t,
            "bias_b": bias_b,
        })

    res = run_bass_kernel_spmd(nc, in_maps, core_ids=list(range(N_CORES)))
    LAST_RESULTS = res

    y = np.concatenate([res.results[c]["y"] for c in range(N_CORES)], axis=1)
    return y.reshape(B_DIM, S_DIM, OUT_F)


# revision 39
# speedup vs baseline: 1.0589x; 1.0589x over previous
"""LoRA Linear (y = x @ W^T + bias + x @ (B@A)^T) on 8 Trainium2 NeuronCores.

Strategy (column-parallel, out_features sharded 8 ways):
  - Each core owns a 512-wide slice of out_features.
  - Split-K mixed precision: the first K8T=16 k-tiles (2048 of 4096
    in_features) run as fp8 e4m3 DoubleRow matmuls (2 k-tiles per
    instruction at 1 cycle/row = 2x bf16 throughput); the remaining 16
    k-tiles run in bf16. Per 128-token chain: 8 DR + 16 bf16 matmuls
    = 24 x 213ns instead of 32 for pure bf16 (~25% less PE time).
    Measured rel err ~1.7e-2 (fp8 quantization noise over half of K),
    within the 2e-2 gate.
  - The LoRA delta is folded into the weight on device before
    quantization: t = W^T + A^T @ B^T (f32); fp8 part: w8 = fp8(t);
    bf16 part: wb = bf16(t), folded in place. The host ships the
    fp8-part weights as bf16 values pre-snapped to the e4m3 grid, so
    the device-side fp8 cast is exact whenever the LoRA delta
    underflows the grid step.
  - W arrives in 4KB-per-partition chunked DMAs (per-k-tile 1KB DMAs
    were packet-overhead-bound and serialized the prologue for ~25us;
    chunking also lets the fold start before the whole 4MB lands).
  - psum layout is [128 tokens, 512 out]; bias is added during PSUM
    eviction; output rows land directly in [tokens, out_shard] layout so
    the host-side gather is a plain concatenate.

Host-side work is layout + dtype encoding only: pack x as [p, T, a, t]
(fp8 plane for the fp8 k-range, bf16 plane for the rest), pre-transpose
W/B slices, broadcast bias; then concatenate the 8 output shards.
"""

import numpy as np
import ml_dtypes

B_DIM, S_DIM = 4, 2048
IN_F = 4096
OUT_F = 4096
RANK = 16
N_CORES = 8
O_SHARD = OUT_F // N_CORES          # 512
TOK = B_DIM * S_DIM                 # 8192
T_TILES = TOK // 128                # 64
K_TILES = IN_F // 128               # 32
K8T = 16                            # k-tiles computed in fp8 (DoubleRow)
KBT = K_TILES - K8T                 # k-tiles computed in bf16
PAIRS = K8T // 2                    # DoubleRow instructions per chain
N_XBUF = 6                          # x-tile pool bufs
W_CHUNK = 4                         # k-tiles per W dma (fold starts early)

_CACHE = {}
LAST_RESULTS = None  # test harness introspection


def _build_nc():
    import concourse.mybir as mybir
    import concourse.tile as tile
    from concourse import bacc

    nc = bacc.Bacc("TRN2", target_bir_lowering=False)
    f32 = mybir.dt.float32
    bf16 = mybir.dt.bfloat16
    f8 = mybir.dt.float8e4
    DR = mybir.MatmulPerfMode.DoubleRow

    x8_d = nc.dram_tensor("x8", (128, T_TILES, K8T, 128), f8,
                          kind="ExternalInput")
    xb_d = nc.dram_tensor("xb", (128, T_TILES, KBT, 128), bf16,
                          kind="ExternalInput")
    w8in_d = nc.dram_tensor("w8in", (128, K8T, O_SHARD), bf16,
                            kind="ExternalInput")
    wb_d = nc.dram_tensor("wb", (128, KBT, O_SHARD), bf16,
                          kind="ExternalInput")
    a_d = nc.dram_tensor("a_t", (RANK, IN_F), bf16, kind="ExternalInput")
    bt_d = nc.dram_tensor("b_t", (RANK, O_SHARD), bf16, kind="ExternalInput")
    bias_d = nc.dram_tensor("bias_b", (128, O_SHARD), f32,
                            kind="ExternalInput")
    y_d = nc.dram_tensor("y", (TOK, O_SHARD), f32, kind="ExternalOutput")

    with tile.TileContext(nc) as tc:
        with (
            tc.tile_pool(name="wpool", bufs=1) as wpool,
            tc.tile_pool(name="wstage", bufs=4) as wstage,
            tc.tile_pool(name="const", bufs=1) as const,
            tc.tile_pool(name="xpool", bufs=N_XBUF) as xpool,
            tc.tile_pool(name="xbpool", bufs=N_XBUF) as xbpool,
            tc.tile_pool(name="opool", bufs=4) as opool,
            tc.tile_pool(name="psum", bufs=4, space="PSUM") as psum_pool,
        ):
            a_sb = const.tile([RANK, IN_F], bf16)
            nc.sync.dma_start(a_sb[:], a_d[:])
            b_sb = const.tile([RANK, O_SHARD], bf16)
            nc.sync.dma_start(b_sb[:], bt_d[:])
            bias_sb = const.tile([128, O_SHARD], f32)
            nc.sync.dma_start(bias_sb[:], bias_d[:])

            # W lands in chunked DMAs (4KB/partition contiguous lines —
            # big enough to avoid packet overhead, small enough that the
            # fold can start before the whole 4MB arrives).
            w8st = wpool.tile([128, K8T, O_SHARD], bf16)
            for c in range(0, K8T, W_CHUNK):
                nc.sync.dma_start(w8st[:, c:c + W_CHUNK, :],
                                  w8in_d[:, c:c + W_CHUNK, :])
            wb_sb = wpool.tile([128, KBT, O_SHARD], bf16)
            for c in range(0, KBT, W_CHUNK):
                nc.sync.dma_start(wb_sb[:, c:c + W_CHUNK, :],
                                  wb_d[:, c:c + W_CHUNK, :])

            # LoRA fold, fp8 k-range (pairs of k-tiles for DoubleRow):
            #   t1 = W^T[k-tile a] + A[:, a*128:(a+1)*128]^T @ B^T   (f32)
            #   w8[pair, slot] = fp8(t1)   (cast on the scalar engine)
            w8_sb = [wpool.tile([128, 2, O_SHARD], f8, tag=f"w8_{p}",
                                name=f"w8_{p}")
                     for p in range(PAIRS)]
            for a in range(K8T):
                p, s = divmod(a, 2)
                pd = psum_pool.tile([128, O_SHARD], f32)
                nc.tensor.matmul(
                    pd[:],
                    a_sb[:, a * 128:(a + 1) * 128],
                    b_sb[:],
                    start=True, stop=True,
                )
                t1 = wstage.tile([128, O_SHARD], f32)
                nc.vector.tensor_add(t1[:], w8st[:, a, :], pd[:])
                nc.scalar.copy(w8_sb[p][:, s, :], t1[:])

            # LoRA fold, bf16 k-range, in place: wb += delta
            for j in range(KBT):
                a = K8T + j
                pd = psum_pool.tile([128, O_SHARD], f32)
                nc.tensor.matmul(
                    pd[:],
                    a_sb[:, a * 128:(a + 1) * 128],
                    b_sb[:],
                    start=True, stop=True,
                )
                nc.vector.tensor_add(wb_sb[:, j, :], wb_sb[:, j, :], pd[:])

            # Main GEMM: psum[128t, 512o] accumulates 8 fp8 DoubleRow
            # matmuls (k-tiles 0..15, two per instruction) + 16 bf16
            # matmuls (k-tiles 16..31) per token tile.
            for t in range(T_TILES):
                x8_sb = xpool.tile([128, K8T, 128], f8)
                nc.sync.dma_start(x8_sb[:], x8_d[:, t, :, :])
                xb_sb = xbpool.tile([128, KBT, 128], bf16)
                nc.sync.dma_start(xb_sb[:], xb_d[:, t, :, :])
                pt = psum_pool.tile([128, O_SHARD], f32)
                for p in range(PAIRS):
                    nc.tensor.matmul(
                        pt[:], x8_sb[:, 2 * p:2 * p + 2, :], w8_sb[p][:],
                        start=(p == 0), stop=False, perf_mode=DR,
                    )
                for j in range(KBT):
                    nc.tensor.matmul(
                        pt[:], xb_sb[:, j, :], wb_sb[:, j, :],
                        start=False, stop=(j == KBT - 1),
                    )
                o_sb = opool.tile([128, O_SHARD], f32)
                nc.vector.tensor_add(o_sb[:], pt[:], bias_sb[:])
                nc.sync.dma_start(y_d[t * 128:(t + 1) * 128, :], o_sb[:])

    nc.compile()
    return nc


def _pack_x(x):
    x2 = np.asarray(x, dtype=np.float32).reshape(TOK, IN_F)
    # x_re[p, T, a, t] = x2[T*128 + t, a*128 + p]
    xr = np.ascontiguousarray(
        x2.reshape(T_TILES, 128, K_TILES, 128).transpose(3, 0, 2, 1))
    x8 = np.ascontiguousarray(xr[:, :, :K8T]).astype(ml_dtypes.float8_e4m3fn)
    xb = np.ascontiguousarray(xr[:, :, K8T:]).astype(ml_dtypes.bfloat16)
    return x8, xb


def kernel(x, weight, A, B, bias):
    global LAST_RESULTS
    from concourse.bass_utils import run_bass_kernel_spmd

    if "nc" not in _CACHE:
        _CACHE["nc"] = _build_nc()
    nc = _CACHE["nc"]

    weight = np.asarray(weight, dtype=np.float32)
    A = np.asarray(A, dtype=np.float32)
    B = np.asarray(B, dtype=np.float32)
    bias = np.asarray(bias, dtype=np.float32)

    x8, xb = _pack_x(x)
    a_t = np.ascontiguousarray(A.astype(ml_dtypes.bfloat16))

    in_maps = []
    for c in range(N_CORES):
        sl = slice(c * O_SHARD, (c + 1) * O_SHARD)
        w_s = weight[sl]                              # (512, 4096)
        # w_re[p, a, o] = w_s[o, a*128 + p]
        w_re = np.ascontiguousarray(
            w_s.T.reshape(K_TILES, 128, O_SHARD).transpose(1, 0, 2))
        # fp8 k-range ships as bf16 snapped to the e4m3 grid (exact in
        # bf16; DVE reads of 8-bit operands measured slower, so the fold
        # consumes bf16 and the scalar engine does the fp8 cast)
        w8in = np.ascontiguousarray(
            w_re[:, :K8T].astype(ml_dtypes.float8_e4m3fn)
            .astype(np.float32).astype(ml_dtypes.bfloat16))
        wb = np.ascontiguousarray(
            w_re[:, K8T:].astype(ml_dtypes.bfloat16))
        b_t = np.ascontiguousarray(B[sl].T.astype(ml_dtypes.bfloat16))
        bias_b = np.ascontiguousarray(
            np.broadcast_to(bias[sl], (128, O_SHARD)))
        in_maps.append({
            "x8": x8,
            "xb": xb,
            "w8in": w8in,
            "wb": wb,
            "a_t": a_t,
            "b_t": b_t,
            "bias_b": bias_b,
        })

    res = run_bass_kernel_spmd(nc, in_maps, core_ids=list(range(N_CORES)))
    LAST_RESULTS = res

    y = np.concatenate([res.results[c]["y"] for c in range(N_CORES)], axis=1)
    return y.reshape(B_DIM, S_DIM, OUT_F)
